# revision 23
# baseline (speedup 1.0000x reference)
"""GraphSAGE mean-concat aggregator on 8 NeuronCores (Bass/Tile).

out = relu(concat(h, mean(nei, axis=1)) @ W.T + b)

Sharding: data-parallel over nodes, W/b replicated, no cross-core
communication. Each core processes 6272 = 49*128 rows so every DMA spans
exactly 128 SBUF partitions (a <128-partition DMA halves every SDMA
engine's beat rate). Cores 0-6 take rows [c*6250, c*6250+6272); core 7
takes the last 6272 rows; the host trims the 22-row overlap on gather.

Per-core kernel (per 128-node tile):
  - DMA nei tile [128, 16*256] as two 1 MB pieces on the sync HWDGE
    queue; h tile [128, 256] + the output store ride the scalar queue
  - VectorE binary-tree sum over the 16 neighbor slices (the 1/16 of the
    mean is folded into the replicated weight host-side)
  - TensorE transposes the 4 [128, 128] chunks of concat(h, agg) via
    identity matmuls (PE->PSUM), ScalarE copies them back to SBUF
  - TensorE accumulates the 4 K=128 chunks of (catT.T @ Wt) into one
    PSUM bank; when b != 0 an extra rank-1 ones x b matmul seeds the
    accumulation with the bias (skipped entirely for b == 0)
  - ScalarE applies ReLU on the PSUM->SBUF copy, DMA out

Measured on trn2 (8 cores concurrent): ~306-380 us per run (the spread
is HBM-stack phase luck between paired cores), vs ~330 us chip-level
HBM roofline for the 941 MB of total traffic.
"""

import numpy as np

import concourse.bacc as bacc
import concourse.mybir as mybir
import concourse.tile as tile
from concourse.bass_utils import run_bass_kernel_spmd
from concourse.masks import make_identity

N_CORES = 8
N = 50000
NB = 16  # neighbors per node
D = 256  # feature dim
OUT = 256
ROWS = N // N_CORES  # 6250 rows of real output per core
NT = 128  # node-tile size
TILES = 49
NS = NT * TILES  # 6272 rows processed per core (22-row overlap on core 7)
F32 = mybir.dt.float32

_CACHED = {}  # with_bias -> compiled program, reused across calls


def _build_program(with_bias):
    nc = bacc.Bacc("TRN2", target_bir_lowering=False, debug=False, num_devices=N_CORES)

    h_d = nc.dram_tensor("h", [NS, D], F32, kind="ExternalInput").ap()
    nei_d = nc.dram_tensor("nei", [NS, NB * D], F32, kind="ExternalInput").ap()
    # host pre-swizzles wt to [128, 4, 256] so this is one contiguous DMA
    wt_d = nc.dram_tensor("wt", [128, 4 * OUT], F32, kind="ExternalInput").ap()
    b_d = nc.dram_tensor("b", [1, OUT], F32, kind="ExternalInput").ap()
    out_d = nc.dram_tensor("out", [NS, OUT], F32, kind="ExternalOutput").ap()

    with tile.TileContext(nc) as tc:
        with (
            tc.tile_pool(name="const", bufs=1) as cpool,
            tc.tile_pool(name="nei", bufs=6) as neipool,
            tc.tile_pool(name="work", bufs=3) as wpool,
            tc.tile_pool(name="io", bufs=4) as iopool,
            tc.tile_pool(name="pst", bufs=4, space="PSUM") as ptpool,
            tc.tile_pool(name="pso", bufs=3, space="PSUM") as popool,
        ):
            ident = cpool.tile([128, 128], F32)
            make_identity(nc, ident[:])
            # const loads ride the scalar queue so the sync queue starts
            # streaming nei immediately
            wt_s = cpool.tile([128, 4, OUT], F32)
            nc.scalar.dma_start(out=wt_s[:], in_=wt_d[:])
            if with_bias:
                ones = cpool.tile([1, 128], F32)
                nc.gpsimd.memset(ones[:], 1.0)
                b_s = cpool.tile([1, OUT], F32)
                nc.scalar.dma_start(out=b_s[:], in_=b_d[:])

            for i in range(TILES):
                r0 = i * NT
                nei_t = neipool.tile([NT, NB * D], F32, tag="nei")
                half = NB * D // 2
                nc.sync.dma_start(
                    out=nei_t[:, :half], in_=nei_d[r0 : r0 + NT, :half]
                )
                nc.sync.dma_start(
                    out=nei_t[:, half:], in_=nei_d[r0 : r0 + NT, half:]
                )
                h_t = iopool.tile([NT, D], F32, tag="h")
                nc.scalar.dma_start(out=h_t[:], in_=h_d[r0 : r0 + NT, :])

                t1 = wpool.tile([NT, 2048], F32, tag="t1")
                nc.vector.tensor_add(t1[:], nei_t[:, :2048], nei_t[:, 2048:])
                t2 = wpool.tile([NT, 1024], F32, tag="t2")
                nc.vector.tensor_add(t2[:], t1[:, :1024], t1[:, 1024:])
                t3 = wpool.tile([NT, 512], F32, tag="t3")
                nc.vector.tensor_add(t3[:], t2[:, :512], t2[:, 512:])
                agg = wpool.tile([NT, D], F32, tag="agg")
                nc.vector.tensor_add(agg[:], t3[:, :256], t3[:, 256:])

                catT = wpool.tile([128, 4, NT], F32, tag="catT")
                srcs = (
                    h_t[:, 0:128],
                    h_t[:, 128:256],
                    agg[:, 0:128],
                    agg[:, 128:256],
                )
                for c, src in enumerate(srcs):
                    pt = ptpool.tile([128, NT], F32, tag="pt")
                    nc.tensor.transpose(pt[:], src, ident[:])
                    nc.scalar.copy(catT[:, c, :], pt[:])

                po = popool.tile([NT, OUT], F32, tag="po")
                if with_bias:
                    nc.tensor.matmul(
                        po[:], ones[:1, :NT], b_s[:1, :], start=True, stop=False
                    )
                for c in range(4):
                    nc.tensor.matmul(
                        po[:],
                        catT[:, c, :],
                        wt_s[:, c, :],
                        start=(c == 0 and not with_bias),
                        stop=(c == 3),
                    )

                o_t = iopool.tile([NT, OUT], F32, tag="o")
                nc.scalar.activation(o_t[:], po[:], mybir.ActivationFunctionType.Relu)
                nc.scalar.dma_start(out=out_d[r0 : r0 + NT, :], in_=o_t[:])

    nc.compile()
    return nc


def _shard_starts():
    starts = [c * ROWS for c in range(N_CORES - 1)]
    starts.append(N - NS)  # core 7 shifted back so its 6272 rows stay in range
    return starts


def _prepare_in_maps(h, nei, W, b):
    h = np.ascontiguousarray(h, dtype=np.float32)
    nei = np.ascontiguousarray(nei, dtype=np.float32)
    W = np.asarray(W, dtype=np.float32)
    b = np.asarray(b, dtype=np.float32)

    wt = np.ascontiguousarray(W.T).astype(np.float32)  # [512, 256]
    wt[D:, :] *= 1.0 / NB  # fold the mean's 1/16 into the agg half
    # swizzle to [p, chunk, o] so the kernel loads it as one contiguous DMA
    wt = np.ascontiguousarray(wt.reshape(4, 128, OUT).transpose(1, 0, 2)).reshape(
        128, 4 * OUT
    )
    b2 = np.ascontiguousarray(b.reshape(1, OUT))

    nei_flat = nei.reshape(N, NB * D)
    in_maps = []
    for s in _shard_starts():
        in_maps.append(
            {
                "h": h[s : s + NS],
                "nei": nei_flat[s : s + NS],
                "wt": wt,
                "b": b2,
            }
        )
    return in_maps


def _run(h, nei, W, b, trace=False):
    with_bias = bool(np.any(np.asarray(b)))
    if with_bias not in _CACHED:
        _CACHED[with_bias] = _build_program(with_bias)
    nc = _CACHED[with_bias]
    in_maps = _prepare_in_maps(h, nei, W, b)
    res = run_bass_kernel_spmd(nc, in_maps, list(range(N_CORES)), trace=trace)
    out = np.empty((N, OUT), dtype=np.float32)
    for c, s in enumerate(_shard_starts()):
        if c < N_CORES - 1:
            out[c * ROWS : c * ROWS + ROWS] = res.results[c]["out"][:ROWS]
        else:
            out[N - ROWS : N] = res.results[c]["out"][NS - ROWS :]
    return out, res


def kernel(**inputs) -> np.ndarray:
    out, _ = _run(inputs["h"], inputs["nei"], inputs["W"], inputs["b"])
    return out


# revision 25
# speedup vs baseline: 1.0028x; 1.0028x over previous
"""GraphSAGE mean-concat aggregator on 8 NeuronCores (Bass/Tile).

out = relu(concat(h, mean(nei, axis=1)) @ W.T + b)

Sharding: data-parallel over nodes, W/b replicated, no cross-core
communication. Each core processes 6272 = 49*128 rows so every DMA spans
exactly 128 SBUF partitions (a <128-partition DMA halves every SDMA
engine's beat rate). Cores 0-6 take rows [c*6250, c*6250+6272); core 7
takes the last 6272 rows; the host trims the 22-row overlap on gather.

Per-core kernel (per 128-node tile):
  - DMA nei tile [128, 16*256] as two 1 MB pieces on the sync HWDGE
    queue; h tile [128, 256] + the output store ride the scalar queue
  - VectorE binary-tree sum over the 16 neighbor slices (the 1/16 of the
    mean is folded into the replicated weight host-side)
  - TensorE transposes the 4 [128, 128] chunks of concat(h, agg) via
    identity matmuls (PE->PSUM), ScalarE copies them back to SBUF
  - TensorE accumulates the 4 K=128 chunks of (catT.T @ Wt) into one
    PSUM bank; when b != 0 an extra rank-1 ones x b matmul seeds the
    accumulation with the bias (skipped entirely for b == 0)
  - ScalarE applies ReLU on the PSUM->SBUF copy, DMA out

Measured on trn2 (8 cores concurrent): ~306-380 us per run (the spread
is HBM-stack phase luck between paired cores), vs ~330 us chip-level
HBM roofline for the 941 MB of total traffic.
"""

import numpy as np

import concourse.bacc as bacc
import concourse.mybir as mybir
import concourse.tile as tile
from concourse.bass_utils import run_bass_kernel_spmd
from concourse.masks import make_identity

N_CORES = 8
N = 50000
NB = 16  # neighbors per node
D = 256  # feature dim
OUT = 256
ROWS = N // N_CORES  # 6250 rows of real output per core
NT = 128  # node-tile size
TILES = 49
NS = NT * TILES  # 6272 rows processed per core (22-row overlap on core 7)
F32 = mybir.dt.float32

_CACHED = {}  # with_bias -> compiled program, reused across calls


def _build_program(with_bias):
    nc = bacc.Bacc("TRN2", target_bir_lowering=False, debug=False, num_devices=N_CORES)

    h_d = nc.dram_tensor("h", [NS, D], F32, kind="ExternalInput").ap()
    nei_d = nc.dram_tensor("nei", [NS, NB * D], F32, kind="ExternalInput").ap()
    # host pre-swizzles wt to [128, 4, 256] so this is one contiguous DMA
    wt_d = nc.dram_tensor("wt", [128, 4 * OUT], F32, kind="ExternalInput").ap()
    b_d = nc.dram_tensor("b", [1, OUT], F32, kind="ExternalInput").ap()
    out_d = nc.dram_tensor("out", [NS, OUT], F32, kind="ExternalOutput").ap()

    with tile.TileContext(nc) as tc:
        with (
            tc.tile_pool(name="const", bufs=1) as cpool,
            tc.tile_pool(name="nei", bufs=6) as neipool,
            tc.tile_pool(name="work", bufs=3) as wpool,
            tc.tile_pool(name="io", bufs=4) as iopool,
            tc.tile_pool(name="pst", bufs=4, space="PSUM") as ptpool,
            tc.tile_pool(name="pso", bufs=3, space="PSUM") as popool,
        ):
            ident = cpool.tile([128, 128], F32)
            make_identity(nc, ident[:])
            # const loads ride the scalar queue so the sync queue starts
            # streaming nei immediately
            wt_s = cpool.tile([128, 4, OUT], F32)
            nc.scalar.dma_start(out=wt_s[:], in_=wt_d[:])
            if with_bias:
                ones = cpool.tile([1, 128], F32)
                nc.gpsimd.memset(ones[:], 1.0)
                b_s = cpool.tile([1, OUT], F32)
                nc.scalar.dma_start(out=b_s[:], in_=b_d[:])

            for i in range(TILES):
                r0 = i * NT
                nei_t = neipool.tile([NT, NB * D], F32, tag="nei")
                half = NB * D // 2
                nc.sync.dma_start(
                    out=nei_t[:, :half], in_=nei_d[r0 : r0 + NT, :half]
                )
                nc.sync.dma_start(
                    out=nei_t[:, half:], in_=nei_d[r0 : r0 + NT, half:]
                )
                h_t = iopool.tile([NT, D], F32, tag="h")
                nc.scalar.dma_start(out=h_t[:], in_=h_d[r0 : r0 + NT, :])

                t1 = wpool.tile([NT, 2048], F32, tag="t1")
                nc.vector.tensor_add(t1[:], nei_t[:, :2048], nei_t[:, 2048:])
                t2 = wpool.tile([NT, 1024], F32, tag="t2")
                nc.vector.tensor_add(t2[:], t1[:, :1024], t1[:, 1024:])
                t3 = wpool.tile([NT, 512], F32, tag="t3")
                nc.vector.tensor_add(t3[:], t2[:, :512], t2[:, 512:])
                agg = wpool.tile([NT, D], F32, tag="agg")
                nc.vector.tensor_add(agg[:], t3[:, :256], t3[:, 256:])

                catT = wpool.tile([128, 4, NT], F32, tag="catT")
                srcs = (
                    h_t[:, 0:128],
                    h_t[:, 128:256],
                    agg[:, 0:128],
                    agg[:, 128:256],
                )
                for c, src in enumerate(srcs):
                    pt = ptpool.tile([128, NT], F32, tag="pt")
                    nc.tensor.transpose(pt[:], src, ident[:])
                    nc.scalar.copy(catT[:, c, :], pt[:])

                po = popool.tile([NT, OUT], F32, tag="po")
                if with_bias:
                    nc.tensor.matmul(
                        po[:], ones[:1, :NT], b_s[:1, :], start=True, stop=False
                    )
                for c in range(4):
                    nc.tensor.matmul(
                        po[:],
                        catT[:, c, :],
                        wt_s[:, c, :],
                        start=(c == 0 and not with_bias),
                        stop=(c == 3),
                    )

                o_t = iopool.tile([NT, OUT], F32, tag="o")
                nc.scalar.activation(o_t[:], po[:], mybir.ActivationFunctionType.Relu)
                nc.scalar.dma_start(out=out_d[r0 : r0 + NT, :], in_=o_t[:])

    nc.compile()
    return nc


def _shard_starts():
    starts = [c * ROWS for c in range(N_CORES - 1)]
    starts.append(N - NS)  # core 7 shifted back so its 6272 rows stay in range
    return starts


def _prepare_in_maps(h, nei, W, b):
    h = np.ascontiguousarray(h, dtype=np.float32)
    nei = np.ascontiguousarray(nei, dtype=np.float32)
    W = np.asarray(W, dtype=np.float32)
    b = np.asarray(b, dtype=np.float32)

    wt = np.ascontiguousarray(W.T).astype(np.float32)  # [512, 256]
    wt[D:, :] *= 1.0 / NB  # fold the mean's 1/16 into the agg half
    # swizzle to [p, chunk, o] so the kernel loads it as one contiguous DMA
    wt = np.ascontiguousarray(wt.reshape(4, 128, OUT).transpose(1, 0, 2)).reshape(
        128, 4 * OUT
    )
    b2 = np.ascontiguousarray(b.reshape(1, OUT))

    nei_flat = nei.reshape(N, NB * D)
    in_maps = []
    for s in _shard_starts():
        in_maps.append(
            {
                "h": h[s : s + NS],
                "nei": nei_flat[s : s + NS],
                "wt": wt,
                "b": b2,
            }
        )
    return in_maps


def _run(h, nei, W, b, trace=False):
    with_bias = bool(np.any(np.asarray(b)))
    if with_bias not in _CACHED:
        _CACHED[with_bias] = _build_program(with_bias)
    nc = _CACHED[with_bias]
    in_maps = _prepare_in_maps(h, nei, W, b)
    res = run_bass_kernel_spmd(nc, in_maps, list(range(N_CORES)), trace=trace)
    out = np.empty((N, OUT), dtype=np.float32)
    for c, s in enumerate(_shard_starts()):
        if c < N_CORES - 1:
            out[c * ROWS : c * ROWS + ROWS] = res.results[c]["out"][:ROWS]
        else:
            out[N - ROWS : N] = res.results[c]["out"][NS - ROWS :]
    return out, res


def kernel(**inputs) -> np.ndarray:
    out, _ = _run(inputs["h"], inputs["nei"], inputs["W"], inputs["b"])
    return out
